# revision 102
# baseline (speedup 1.0000x reference)
"""Alternating band/temporal BiLSTM stack (BSRNN-style) on 8 TRN2 NeuronCores.

Sharding: 8-way over the per-module LSTM-instance axis (temporal module: 4
bands/core over seq T=64; band module: 8 frames/core over seq K=32); batch
B=4 stays together. X is exchanged with an AllToAll at the three internal
module boundaries; groupnorm partial stats ride along in the exchanged
buffer. The last band module's per-core output slices are assembled on the
host.

GroupNorm folding: gamma is folded into W_ih on the host (W' = W_ih
diag(gamma)), so the device only needs the per-batch scale rs_b (applied
once to the imported X) and a per-module additive gate correction
corr = W_ih@beta + b_ih + b_hh - m_b * W_ih@gamma, assembled on-device from
two uploaded static tensors (base, -W@gamma) and the runtime scalar
m_b = mu_b*rs_b. No normalized-X staging pass and no separate input
projection pass exist: each step's gate PSUM accumulates
[16 W'_ih@x-column matmuls] + [identity@corr deposit] + [16 W_hh@h matmuls]
directly (PE matmuls are cheap; this removes all DVE/Act staging work).

The skip path reads straight out of the A2A receive buffer via a strided
view (layer-0 temporal reads the raw uploaded x).

Step chain per direction: PSUM gates -> one sigmoid (g-gate rows doubled on
the host so tanh(g)=2*sigmoid(2g)-1 rides the same op) -> t2=f*c (gpsimd) |
t1=(A-0.5)*i (DVE) -> c=2*t1+t2 (DVE) -> tanh(c) (Act) -> h=o*tanh (gpsimd).
DVE program order is t1_d, c_d per direction so c never queues behind the
other direction's t1.

Precision: weights, h, corr, and the A2A payload in bf16; PSUM/c in fp32.
"""

import numpy as np

import concourse.bass as bass  # noqa: F401
import concourse.bacc as bacc
import concourse.mybir as mybir
from concourse import tile
from concourse.bass_utils import run_bass_kernel_spmd

R = 8            # cores
B = 4            # batch
K = 32           # bands (KMOD)
T = 64           # frames
N = 128          # features
G = 512          # 4N gates
L = 2            # layers
EPS = 1e-5
NT = K // R      # temporal instances per core
NB = T // R      # band instances per core
M_STAT = K * T * N
CW = 136         # per-destination chunk: 128 data cols + 8 stats cols

FP = mybir.dt.float32
WDT = mybir.dt.bfloat16
HDT = WDT

GATE_PERM = [0, 1, 3, 2]  # torch (i,f,g,o) -> slots (i,f,o,g)


# ---------------------------------------------------------------------------
# device graph
# ---------------------------------------------------------------------------

def build_nc():
    nc = bacc.Bacc(None, target_bir_lowering=False, debug=False, num_devices=R)

    xs = nc.dram_tensor("xs", [N, NT, T, B], FP, kind="ExternalInput")
    xsc0 = nc.dram_tensor("xsc0", [N, NT, T, B], WDT, kind="ExternalInput")
    corr0 = nc.dram_tensor("corr0", [N, 2, 4, NT, B], WDT,
                           kind="ExternalInput")
    twi = nc.dram_tensor("twi", [L, N, NT, 2, G], WDT, kind="ExternalInput")
    twh = nc.dram_tensor("twh", [L, N, NT, 2, G], HDT, kind="ExternalInput")
    tbase = nc.dram_tensor("tbase", [L, N, 2, 4, NT], FP,
                           kind="ExternalInput")
    twg = nc.dram_tensor("twg", [L, N, 2, 4, NT], FP, kind="ExternalInput")
    tfcw = nc.dram_tensor("tfcw", [L, N, NT, 2, N], WDT, kind="ExternalInput")
    tfcb = nc.dram_tensor("tfcb", [L, N, NT], FP, kind="ExternalInput")
    bwi = nc.dram_tensor("bwi", [L, N, NB, 2, G], WDT, kind="ExternalInput")
    bwh = nc.dram_tensor("bwh", [L, N, NB, 2, G], HDT, kind="ExternalInput")
    bbase = nc.dram_tensor("bbase", [L, N, 2, 4, NB], FP,
                           kind="ExternalInput")
    bwg = nc.dram_tensor("bwg", [L, N, 2, 4, NB], FP, kind="ExternalInput")
    bfcw = nc.dram_tensor("bfcw", [L, N, NB, 2, N], WDT, kind="ExternalInput")
    bfcb = nc.dram_tensor("bfcb", [L, N, NB], FP, kind="ExternalInput")
    out_ext = nc.dram_tensor("out", [N, NB, K, B], FP, kind="ExternalOutput")

    rg = [list(range(R))]

    with tile.TileContext(nc) as tc:
        with (
            tc.tile_pool(name="dram", bufs=1, space="DRAM") as dram,
            tc.tile_pool(name="const", bufs=1) as cpool,
            tc.tile_pool(name="wt", bufs=2) as wtp,
            tc.tile_pool(name="wb", bufs=2) as wbp,
            tc.tile_pool(name="xp", bufs=1) as xp,
            tc.tile_pool(name="hcp", bufs=2) as hcp,
            tc.tile_pool(name="gts", bufs=24) as gts,
            tc.tile_pool(name="smp", bufs=2) as smp,
            tc.tile_pool(name="cst", bufs=1) as cst,
            tc.tile_pool(name="ps_g", bufs=2, space="PSUM") as ps_g,
            tc.tile_pool(name="ps_f", bufs=3, space="PSUM") as ps_f,
            tc.tile_pool(name="ps_s", bufs=1, space="PSUM") as ps_s,
        ):
            # A2A bounce/output buffers for the 3 internal boundaries
            bnc = [dram.tile([R, N, CW], WDT, name=f"bnc{i}") for i in range(3)]
            gth = [dram.tile([R, N, CW], WDT, name=f"gth{i}") for i in range(3)]

            ones = cpool.tile([N, N], WDT)
            nc.vector.memset(ones[:], 1.0)
            from concourse.masks import make_identity
            ident = cpool.tile([N, N], WDT)
            make_identity(nc, ident[:])
            zeros = cpool.tile([N, B], HDT)
            nc.vector.memset(zeros[:], 0.0)
            eps_t = cpool.tile([N, 1], FP)
            nc.vector.memset(eps_t[:], float(EPS))
            dmy = cpool.tile([N, 1], FP)

            def act_preload():
                # tiny sigmoid forces the sigmoid/tanh act-func-set load NOW
                # (off the critical path) instead of right before the next
                # real sigmoid; the dbg DMA keeps it live through DCE
                nc.scalar.activation(dmy[:], eps_t[:],
                                     mybir.ActivationFunctionType.Sigmoid)

            act_preload()

            # ---------------- helpers ----------------

            def stats_prep(gst):
                """gst [N, R, 8] partial moments (already /M on the sender)
                -> (rs [N,B], m [N,B])."""
                ps = ps_s.tile([N, 8], FP, tag="ps_tot")
                for r in range(R):
                    nc.tensor.matmul(ps[:], ones[:], gst[:, r],
                                     start=(r == 0), stop=(r == R - 1))
                mom = smp.tile([N, 8], FP, tag="st_mom")
                nc.vector.tensor_copy(mom[:], ps[:])
                msq = smp.tile([N, 4], FP, tag="st_msq")
                nc.vector.tensor_tensor(msq[:], mom[:, 0:4], mom[:, 0:4],
                                        op=mybir.AluOpType.mult)
                var = smp.tile([N, 4], FP, tag="st_var")
                nc.vector.tensor_tensor(var[:], mom[:, 4:8], msq[:],
                                        op=mybir.AluOpType.subtract)
                sd = smp.tile([N, 4], FP, tag="st_sd")
                nc.scalar.activation(sd[:], var[:],
                                     mybir.ActivationFunctionType.Sqrt,
                                     bias=eps_t[:, 0:1])
                rs = smp.tile([N, 4], FP, tag="st_rs")
                nc.vector.reciprocal(rs[:], sd[:])
                m = smp.tile([N, 4], FP, tag="st_m")
                nc.vector.tensor_tensor(m[:], mom[:, 0:4], rs[:],
                                        op=mybir.AluOpType.mult)
                return rs, m

            def make_corr(base_t, wg_t, m, ni, tag):
                """corr[N, 2, 4, ni, B] (bf16) = base + m_b * (-W@gamma).
                base/wg uploaded as [N, 2, 4, ni] so (c j) merges to keep
                every operand <=3D for the verifier."""
                corr = smp.tile([N, 2, 4, ni, B], WDT, tag=tag)
                wgv = wg_t[:].rearrange("p d c j -> p d (c j)")
                bsv = base_t[:].rearrange("p d c j -> p d (c j)")
                for b in range(B):
                    nc.vector.scalar_tensor_tensor(
                        corr[:, :, :, :, b].rearrange("p d c j -> p d (c j)"),
                        wgv, m[:, b:b + 1], bsv,
                        op0=mybir.AluOpType.mult, op1=mybir.AluOpType.add)
                return corr

            def data_view(gsb, kind):
                """Received [N, R, CW] -> 5-dim X view (no copy).
                t: [p, kk(inst), q, tt, b] with seq t=(q,tt);
                b: [p, q, kk, tt(inst), b] with seq k=(q,kk)."""
                d = gsb[:, :, 0:128]
                if kind == "t":
                    return d.rearrange("p q (kk tt b) -> p kk q tt b",
                                       kk=NT, tt=NB)
                return d.rearrange("p q (kk tt b) -> p q kk tt b",
                                   kk=NT, tt=NB)

            def skip_slicer(xsk, kind):
                """skip_of(j) -> dense [N, S, B] instance view."""
                if kind == "t":
                    return lambda j: xsk[:, j]
                return lambda j: xsk[:, :, j]

            def import_scale(gsb, rs, kind, tagk, tagc):
                """Materialize recv data (dense bf16 skip tile) and the
                rs-scaled copy, per seq range with outer quarters first.
                Copies on DVE (middle on Act); scales alternate DVE/Act
                per batch so neither engine serializes the whole chain."""
                shape = [N, NT, T, B] if kind == "t" else [N, K, NB, B]
                w = NB if kind == "t" else NT
                xsk = xp.tile(shape, WDT, tag=tagk)
                xsc = xp.tile(shape, WDT, tag=tagc)
                d = gsb[:, :, 0:128]
                for lo, hi in ((0, 2), (6, 8), (2, 6)):
                    src = d[:, lo:hi].rearrange(
                        "p q (kk ttb) -> p kk q ttb" if kind == "t"
                        else "p q (kk ttb) -> p q kk ttb", kk=NT)
                    if kind == "t":
                        dst = xsk[:, :, lo * NB:hi * NB, :].rearrange(
                            "p kk (q tt) b -> p kk q (tt b)", tt=NB)
                    else:
                        dst = xsk[:, lo * NT:hi * NT, :, :].rearrange(
                            "p (q kk) tt b -> p q kk (tt b)", kk=NT)
                    if lo == 2:
                        nc.scalar.activation(
                            dst, src, mybir.ActivationFunctionType.Copy)
                    else:
                        nc.vector.tensor_copy(dst, src)
                    # one broadcast multiply per range (all batches): a
                    # single writer per region keeps each step matmul's
                    # wait list at one semaphore instead of four
                    if kind == "t":
                        dv = xsc[:, :, lo * w:hi * w, :]
                        sv = xsk[:, :, lo * w:hi * w, :]
                    else:
                        dv = xsc[:, lo * w:hi * w, :, :]
                        sv = xsk[:, lo * w:hi * w, :, :]
                    rsb = (rs[:].unsqueeze(1).unsqueeze(1)
                           .broadcast_to(list(dv.shape)))
                    nc.vector.tensor_tensor(dv, sv, rsb,
                                            op=mybir.AluOpType.mult)
                return xsk, xsc

            def module(kind, xsc, skip_of, corr, wi_t, wh_t, fw_t, fb_t,
                       contrib, out_dma=None, cmaj=False):
                """One module. contrib: [N, NT, T, B] (t), [N, K, NB, B]
                (band), or [N, NB, K, B] (band with cmaj=True).
                skip_of(j) -> dense [N, S, B] view."""
                NI = NT if kind == "t" else NB
                S = T if kind == "t" else K
                W = NI * B

                def inst(ap, j):  # [N, S, B] contrib view for instance j
                    return ap[:, j] if (kind == "t" or cmaj) else ap[:, :, j]

                def xcol(j, pos):  # [N, B] input column
                    return (xsc[:, j, pos] if kind == "t"
                            else xsc[:, pos, j])

                h_dir = [hcp.tile([N, S, NI, B], HDT, tag=f"h{d}",
                                  name=f"h{d}") for d in range(2)]
                c_st = [cst.tile([N, 2, W], FP, tag=f"cs{p}", name=f"cs{p}")
                        for p in range(2)]
                nc.vector.memset(c_st[1][:], 0.0)

                def fc_range(j, p0, p1):
                    pf = ps_f.tile([N, p1 - p0, B], FP, tag="pf", name="pf")
                    for d in range(2):
                        nc.tensor.matmul(pf[:], fw_t[:, j, d],
                                         h_dir[d][:, p0:p1, j, :],
                                         start=(d == 0), stop=(d == 1))
                    nc.vector.scalar_tensor_tensor(
                        inst(contrib, j)[:, p0:p1, :], pf[:],
                        fb_t[:, j:j + 1], skip_of(j)[:, p0:p1, :],
                        op0=mybir.AluOpType.add, op1=mybir.AluOpType.add)

                def quarters(ap):
                    """contiguous [p, S, B] -> [p, 2, S/4*B]: outer
                    quarters via a step-slice on the quarter axis."""
                    return (ap.rearrange("p (x tt) b -> p x tt b",
                                         tt=S // 4)
                            .rearrange("p x tt b -> p x (tt b)")[:, 0:4:3])

                def fc_outer(j):
                    """FC for both outer quarters off one PSUM group; one
                    merged stt for temporal (contiguous views), two for
                    band (strided instance views can't merge (tt b))."""
                    q = S // 4
                    pf = ps_f.tile([N, 2, q, B], FP, tag="pf", name="pf")
                    first = True
                    for d in range(2):
                        for hh, (r0, r1) in enumerate(((0, q), (S - q, S))):
                            nc.tensor.matmul(pf[:, hh], fw_t[:, j, d],
                                             h_dir[d][:, r0:r1, j, :],
                                             start=first,
                                             stop=(d == 1 and hh == 1))
                            first = False
                    if kind == "t":
                        nc.vector.scalar_tensor_tensor(
                            quarters(inst(contrib, j)),
                            pf[:].rearrange("p x tt b -> p x (tt b)"),
                            fb_t[:, j:j + 1], quarters(skip_of(j)),
                            op0=mybir.AluOpType.add,
                            op1=mybir.AluOpType.add)
                    else:
                        for hh, (r0, r1) in enumerate(((0, q), (S - q, S))):
                            nc.vector.scalar_tensor_tensor(
                                inst(contrib, j)[:, r0:r1, :], pf[:, hh],
                                fb_t[:, j:j + 1], skip_of(j)[:, r0:r1, :],
                                op0=mybir.AluOpType.add,
                                op1=mybir.AluOpType.add)

                for s in range(S):
                    acts = {}
                    for d in range(2):
                        pos = s if d == 0 else S - 1 - s
                        pg = ps_g.tile([N, 4, NI, B], FP, tag=f"pg{d}",
                                       name=f"pg{d}")
                        # start=True on the first matmul zeroes the whole
                        # 2KB PSUM zero-region (the bank), so later matmuls
                        # accumulate-from-zero without per-region starts
                        first = True
                        for j in range(NI):
                            xc = xcol(j, pos)
                            for c in range(4):
                                nc.tensor.matmul(
                                    pg[:, c, j],
                                    wi_t[:, j, d, c * N:(c + 1) * N],
                                    xc, start=first, stop=False,
                                    skip_group_check=True)
                                first = False
                        nc.tensor.matmul(pg[:], ident[:], corr[:, d],
                                         start=False, stop=(s == 0),
                                         skip_group_check=True)
                        if s > 0:
                            for j in range(NI):
                                prv = s - 1 if d == 0 else S - s
                                hprev = h_dir[d][:, prv, j, :]
                                for c in range(4):
                                    nc.tensor.matmul(
                                        pg[:, c, j],
                                        wh_t[:, j, d, c * N:(c + 1) * N],
                                        hprev, start=False,
                                        stop=(j == NI - 1 and c == 3),
                                        skip_group_check=True)
                        act = gts.tile([N, 4, W], FP, tag=f"act{d}",
                                       name=f"act{d}")
                        pgv = pg[:].rearrange("p c j b -> p c (j b)")
                        # g-chunk weights/corr are pre-doubled on the host, so
                        # one sigmoid covers all four chunks; tanh(g)=2*A-1
                        nc.scalar.activation(act[:], pgv[:],
                                             mybir.ActivationFunctionType.Sigmoid)
                        acts[d] = act
                    # c = 2*(A-0.5)*i + f*c_prev; t2/t1/c back-to-back on DVE
                    # (no cross-engine hop), grouped per direction so c_d
                    # never queues behind the other direction's ops
                    for d in range(2):
                        t2 = gts.tile([N, W], FP, tag=f"t2{d}", name=f"t2{d}")
                        nc.vector.tensor_tensor(t2[:], acts[d][:, 1],
                                                c_st[1 - s % 2][:, d],
                                                op=mybir.AluOpType.mult)
                        t1 = gts.tile([N, W], FP, tag=f"t1{d}", name=f"t1{d}")
                        nc.vector.scalar_tensor_tensor(
                            t1[:], acts[d][:, 3], 0.5, acts[d][:, 0],
                            op0=mybir.AluOpType.subtract,
                            op1=mybir.AluOpType.mult)
                        nc.vector.scalar_tensor_tensor(
                            c_st[s % 2][:, d], t1[:], 2.0, t2[:],
                            op0=mybir.AluOpType.mult, op1=mybir.AluOpType.add)
                    tch = gts.tile([N, 2, W], FP, tag="tc")
                    for d in range(2):
                        nc.scalar.activation(tch[:, d], c_st[s % 2][:, d],
                                             mybir.ActivationFunctionType.Tanh)
                    for d in range(2):
                        pos = s if d == 0 else S - 1 - s
                        nc.gpsimd.tensor_tensor(
                            h_dir[d][:, pos].rearrange("p j b -> p (j b)"),
                            acts[d][:, 2], tch[:, d], op=mybir.AluOpType.mult)
                    # FC middle half dripped behind the recurrence
                    fj = s - (3 * S // 4 - 1)
                    if 0 <= fj < NI:
                        fc_range(fj, S // 4, 3 * S // 4)

                # FC + bias + skip for the outer quarters
                for j in range(NI):
                    fc_outer(j)
                    if out_dma is not None:
                        out_dma(j)

            def partial_stats(contrib):
                """[N, 8] per-partition moments/M (b: 0..3 mean part,
                4..7 E[x^2] part): 1/M folded into the activation scale
                (1/sqrt(M) for the squares; M = 2^18 so both are exact)."""
                stp = smp.tile([N, 8], FP, tag="stp")
                # squares+accum on Act; plain sums on DVE tensor_reduce so
                # the two halves run on different engines concurrently.
                # Separate scratch slices per op (a shared scratch would
                # WAW-chain the activations at full effect latency).
                scr = smp.tile([N, 4, 256], WDT, tag="scr")
                raw = smp.tile([N, 4], FP, tag="st_raw")
                for b in range(B):
                    cv = contrib[:, :, :, b]
                    svb = scr[:, b].rearrange("p (a s) -> p a s",
                                              a=cv.shape[1])
                    nc.vector.tensor_reduce(raw[:, b:b + 1], cv,
                                            axis=mybir.AxisListType.XY,
                                            op=mybir.AluOpType.add)
                    nc.scalar.activation(svb, cv,
                                         mybir.ActivationFunctionType.Square,
                                         scale=1.0 / (M_STAT ** 0.5),
                                         accum_out=stp[:, 4 + b:5 + b])
                nc.vector.tensor_scalar(stp[:, 0:4], raw[:], 1.0 / M_STAT,
                                        None, op0=mybir.AluOpType.mult)
                return stp

            def export_data(contrib, bt, kind):
                """contrib data cols -> bounce rows; DMAs don't wait on
                stats (those ride a separate small DMA)."""
                ex = xp.tile([N, R, CW], WDT, tag="ex")
                if kind == "t":
                    src = contrib[:].rearrange(
                        "p kk (q tt) b -> p q kk (tt b)", q=R)
                else:
                    src = contrib[:].rearrange(
                        "p (q kk) tt b -> p q kk (tt b)", q=R)
                nc.vector.tensor_copy(
                    ex[:, :, 0:128].rearrange("p q (kk tb) -> p q kk tb",
                                              kk=NT), src)
                for q4 in range(0, R, 4):
                    nc.sync.dma_start(
                        out=bt[q4:q4 + 4, :, 0:128].rearrange(
                            "q p c -> p q c"),
                        in_=ex[:, q4:q4 + 4, 0:128])
                return ex

            def export_stats(ex, stp, bt):
                nc.vector.tensor_copy(
                    ex[:, :, 128:],
                    stp[:].unsqueeze(1).broadcast_to([N, R, 8]))
                nc.sync.dma_start(
                    out=bt[:, :, 128:].rearrange("q p c -> p q c"),
                    in_=ex[:, :, 128:])

            def a2a(i):
                nc.gpsimd.collective_compute(
                    "AllToAll", mybir.AluOpType.bypass, replica_groups=rg,
                    ins=[bnc[i].opt()], outs=[gth[i].opt()])

            def import_g(i):
                """gathered [R, N, CW] -> SBUF; stats cols first so the
                scale/corr chain overlaps the big data import."""
                g = gth[i]
                gst = xp.tile([N, R, 8], WDT, tag=f"gst{i}", name=f"gst{i}")
                nc.sync.dma_start(
                    out=gst[:],
                    in_=g[:].rearrange("q p c -> p q c")[:, :, 128:])
                gsb = xp.tile([N, R, CW], WDT, tag=f"gsb{i}", name=f"gsb{i}")
                for k, q4 in enumerate(range(0, R, 4)):
                    eng = nc.scalar if k == 0 else nc.sync
                    eng.dma_start(
                        out=gsb[:, q4:q4 + 4],
                        in_=g[q4:q4 + 4].rearrange("q p c -> p q c"))
                return gsb, gst

            def load_t(li):
                wi = wtp.tile([N, NT, 2, G], WDT, tag="twi")
                wh = wtp.tile([N, NT, 2, G], HDT, tag="twh")
                fw = wtp.tile([N, NT, 2, N], WDT, tag="tfw")
                bs = wtp.tile([N, 2, 4, NT], FP, tag="tbs")
                wg = wtp.tile([N, 2, 4, NT], FP, tag="twgt")
                fb = wtp.tile([N, NT], FP, tag="tfb")
                for dst, src in ((wi, twi), (wh, twh), (fw, tfcw),
                                 (bs, tbase), (wg, twg), (fb, tfcb)):
                    nc.sync.dma_start(out=dst[:], in_=src[li])
                return wi, wh, fw, fb, bs, wg

            def load_b(li):
                wi = wbp.tile([N, NB, 2, G], WDT, tag="bwi")
                wh = wbp.tile([N, NB, 2, G], HDT, tag="bwh")
                fw = wbp.tile([N, NB, 2, N], WDT, tag="bfw")
                bs = wbp.tile([N, 2, 4, NB], FP, tag="bbs")
                wg = wbp.tile([N, 2, 4, NB], FP, tag="bwgt")
                fb = wbp.tile([N, NB], FP, tag="bfb")
                for dst, src in ((wi, bwi), (wh, bwh), (fw, bfcw),
                                 (bs, bbase), (wg, bwg), (fb, bfcb)):
                    nc.sync.dma_start(out=dst[:], in_=src[li])
                return wi, wh, fw, fb, bs, wg

            # ---------------- the 4 modules ----------------

            # T0: projections from uploaded rs-scaled x; corr precomputed.
            # Critical-path loads (xsc0, corr0, wi halves) split across the
            # SP and Activation HWDGE queues; the rest follows.
            xsc0_t = xp.tile([N, NT, T, B], WDT, tag="xsc0")
            nc.sync.dma_start(out=xsc0_t[:], in_=xsc0[:])
            corr0_t = smp.tile([N, 2, 4, NT, B], WDT, tag="corr0")
            nc.scalar.dma_start(out=corr0_t[:], in_=corr0[:])
            wi = wtp.tile([N, NT, 2, G], WDT, tag="twi")
            nc.sync.dma_start(out=wi[:, :, 0], in_=twi[0, :, :, 0])
            nc.scalar.dma_start(out=wi[:, :, 1], in_=twi[0, :, :, 1])
            wh = wtp.tile([N, NT, 2, G], HDT, tag="twh")
            nc.sync.dma_start(out=wh[:, :, 0], in_=twh[0, :, :, 0])
            nc.scalar.dma_start(out=wh[:, :, 1], in_=twh[0, :, :, 1])
            xs_t = xp.tile([N, NT, T, B], FP, tag="xs")
            nc.sync.dma_start(out=xs_t[:], in_=xs[:])
            fw = wtp.tile([N, NT, 2, N], WDT, tag="tfw")
            nc.scalar.dma_start(out=fw[:], in_=tfcw[0])
            fb = wtp.tile([N, NT], FP, tag="tfb")
            nc.scalar.dma_start(out=fb[:], in_=tfcb[0])
            # prefetch every later module's weights now: emitted here,
            # their DMAs run during the loops instead of queueing behind
            # boundary export DMAs on the in-order SP queue
            wB0 = load_b(0)
            wT1 = load_t(1)
            wB1 = load_b(1)
            ct = cst.tile([N, NT, T, B], FP, tag="ct")
            module("t", xsc0_t, lambda j: xs_t[:, j], corr0_t,
                   wi, wh, fw, fb, ct)
            ex = export_data(ct, bnc[0], "t")
            export_stats(ex, partial_stats(ct), bnc[0])
            a2a(0)

            # B0
            gsb, gst = import_g(0)
            wi, wh, fw, fb, bs, wg = wB0
            rs, m = stats_prep(gst)
            corr = make_corr(bs, wg, m, NB, "corrb")
            xsk, xb = import_scale(gsb, rs, "b", "xk0", "xb0")
            cb = cst.tile([N, K, NB, B], FP, tag="cb")
            module("b", xb, skip_slicer(xsk, "b"), corr, wi, wh, fw, fb, cb)
            ex = export_data(cb, bnc[1], "b")
            export_stats(ex, partial_stats(cb), bnc[1])
            a2a(1)

            # T1
            gsb, gst = import_g(1)
            wi, wh, fw, fb, bs, wg = wT1
            rs, m = stats_prep(gst)
            corr = make_corr(bs, wg, m, NT, "corrt")
            xsk, xt1 = import_scale(gsb, rs, "t", "xk1", "xt1")
            ct1 = cst.tile([N, NT, T, B], FP, tag="ct")
            module("t", xt1, skip_slicer(xsk, "t"), corr, wi, wh, fw, fb,
                   ct1)
            ex = export_data(ct1, bnc[2], "t")
            export_stats(ex, partial_stats(ct1), bnc[2])
            a2a(2)

            # B1: stream the output DMA per finished instance
            gsb, gst = import_g(2)
            wi, wh, fw, fb, bs, wg = wB1
            rs, m = stats_prep(gst)
            corr = make_corr(bs, wg, m, NB, "corrb")
            xsk, xb1 = import_scale(gsb, rs, "b", "xk2", "xb1")
            # instance-major contrib so each instance's rows are contiguous
            # and its output DMA streams right after its last FC
            cb1 = cst.tile([N, NB, K, B], FP, tag="cb1")

            def out_dma(j):
                eng = nc.sync if j % 2 == 0 else nc.scalar
                eng.dma_start(out=out_ext[:, j], in_=cb1[:, j])

            module("b", xb1, skip_slicer(xsk, "b"), corr, wi, wh, fw, fb,
                   cb1, out_dma=out_dma, cmaj=True)

    nc.finalize()
    return nc


# ---------------------------------------------------------------------------
# host side
# ---------------------------------------------------------------------------

_NC_CACHE = {}


def _wdt_np(a):
    import ml_dtypes
    return np.asarray(a, dtype=ml_dtypes.bfloat16)


def _perm_double(w4):
    """w4 [..., 4, X]: apply GATE_PERM on axis -2 and double the g slot."""
    w4 = np.asarray(w4[..., GATE_PERM, :]).copy()
    w4[..., 3, :] *= 2.0
    return w4


def _prep_lstm_w(w, sl, gamma=None):
    """w: [L, I, 2, G, N] full -> lhsT [L, N, ni, 2, G] for instances sl.
    gamma [L, N] is folded in along the contraction axis when given."""
    ws = np.asarray(w)[:, sl].astype(np.float64)  # [L, ni, 2, G, N]
    ni = ws.shape[1]
    if gamma is not None:
        ws = ws * np.asarray(gamma, np.float64)[:, None, None, None, :]
    ws = ws.reshape(L, ni, 2, 4, N * N)
    ws = _perm_double(ws).reshape(L, ni, 2, 4, N, N)
    # (l, j, d, c, g, n) -> (l, n, j, d, c, g)
    ws = ws.transpose(0, 5, 1, 2, 3, 4).reshape(L, N, ni, 2, G)
    return np.ascontiguousarray(ws.astype(np.float32))


def _prep_base_wg(w_ih, b_ih, b_hh, beta, gamma, sl):
    """base = perm/doubled (W@beta + b_ih + b_hh); wgneg = -perm/doubled
    W@gamma. Returns [L, N, ni, 2, 4] fp32 each."""
    w = np.asarray(w_ih)[:, sl].astype(np.float64)      # [L, ni, 2, G, N]
    ni = w.shape[1]
    wb = np.einsum("ljdgn,ln->ljdg", w, np.asarray(beta, np.float64))
    wg = np.einsum("ljdgn,ln->ljdg", w, np.asarray(gamma, np.float64))
    bsum = (np.asarray(b_ih)[:, sl] + np.asarray(b_hh)[:, sl]).astype(
        np.float64)                                     # [L, ni, 2, G]
    base = (wb + bsum).reshape(L, ni, 2, 4, N)
    wgn = (-wg).reshape(L, ni, 2, 4, N)
    base = _perm_double(base)
    wgn = _perm_double(wgn)
    # (l, j, d, c, g) -> (l, g, d, c, j)
    base = base.transpose(0, 4, 2, 3, 1)
    wgn = wgn.transpose(0, 4, 2, 3, 1)
    return (np.ascontiguousarray(base.astype(np.float32)),
            np.ascontiguousarray(wgn.astype(np.float32)))


def _prep_fcw(w, sl):
    ws = np.asarray(w)[:, sl]                     # [L, ni, N, 2N]
    ni = ws.shape[1]
    ws = ws.reshape(L, ni, N, 2, N)
    # (l, j, no, d, dk) -> (l, dk, j, d, no)
    return np.ascontiguousarray(ws.transpose(0, 4, 1, 3, 2))


def kernel(x, tw_ih, tw_hh, tb_ih, tb_hh, tfc_w, tfc_b, tgn_g, tgn_b,
           bw_ih, bw_hh, bb_ih, bb_hh, bfc_w, bfc_b, bgn_g, bgn_b):
    x = np.asarray(x, dtype=np.float32)

    if "nc" not in _NC_CACHE:
        _NC_CACHE["nc"] = build_nc()
    nc = _NC_CACHE["nc"]

    # layer-0 temporal groupnorm: exact host stats
    mu = x.mean(axis=(1, 2, 3))                    # [B]
    var = x.var(axis=(1, 2, 3))                    # [B]
    rs0 = 1.0 / np.sqrt(var + EPS)                 # [B]
    m0 = mu * rs0                                  # [B]
    xsc0_full = x * rs0[:, None, None, None]       # [B, K, T, N]

    tgn_g = np.asarray(tgn_g, np.float32)
    tgn_b = np.asarray(tgn_b, np.float32)
    bgn_g = np.asarray(bgn_g, np.float32)
    bgn_b = np.asarray(bgn_b, np.float32)

    in_maps = []
    for r in range(R):
        slt = slice(4 * r, 4 * r + 4)
        slb = slice(8 * r, 8 * r + 8)
        xs = np.ascontiguousarray(
            x[:, slt].transpose(3, 1, 2, 0)).astype(np.float32)  # [N,NT,T,B]
        xsc0 = np.ascontiguousarray(
            xsc0_full[:, slt].transpose(3, 1, 2, 0))             # [N,NT,T,B]
        tbase_r, twg_r = _prep_base_wg(tw_ih, tb_ih, tb_hh,
                                       tgn_b, tgn_g, slt)
        bbase_r, bwg_r = _prep_base_wg(bw_ih, bb_ih, bb_hh,
                                       bgn_b, bgn_g, slb)
        # corr0 = base + m_b * wgneg for layer-0 temporal: [N, 2, 4, NT, B]
        corr0_r = np.ascontiguousarray(
            tbase_r[0][..., None]
            + twg_r[0][..., None] * m0[None, None, None, :])
        tfcb_r = np.ascontiguousarray(
            np.asarray(tfc_b)[:, slt].transpose(0, 2, 1)).astype(np.float32)
        bfcb_r = np.ascontiguousarray(
            np.asarray(bfc_b)[:, slb].transpose(0, 2, 1)).astype(np.float32)
        in_maps.append({
            "xs": xs,
            "xsc0": _wdt_np(xsc0),
            "corr0": _wdt_np(corr0_r),
            "twi": _wdt_np(_prep_lstm_w(tw_ih, slt, gamma=tgn_g)),
            "twh": _wdt_np(_prep_lstm_w(tw_hh, slt)),
            "tbase": tbase_r,
            "twg": twg_r,
            "tfcw": _wdt_np(_prep_fcw(tfc_w, slt)),
            "tfcb": tfcb_r,
            "bwi": _wdt_np(_prep_lstm_w(bw_ih, slb, gamma=bgn_g)),
            "bwh": _wdt_np(_prep_lstm_w(bw_hh, slb)),
            "bbase": bbase_r,
            "bwg": bwg_r,
            "bfcw": _wdt_np(_prep_fcw(bfc_w, slb)),
            "bfcb": bfcb_r,
        })

    global _LAST_IN_MAPS
    _LAST_IN_MAPS = in_maps
    res = run_bass_kernel_spmd(nc, in_maps, core_ids=list(range(R)))
    outs = res.results

    full = np.zeros((B, K, T, N), dtype=np.float32)
    for r in range(R):
        o = np.asarray(outs[r]["out"]).reshape(N, NB, K, B)
        full[:, :, 8 * r:8 * r + 8, :] = o.transpose(3, 2, 1, 0)
    return full[:, :K - 2]


# revision 104
# speedup vs baseline: 1.0035x; 1.0035x over previous
"""Alternating band/temporal BiLSTM stack (BSRNN-style) on 8 TRN2 NeuronCores.

Sharding: 8-way over the per-module LSTM-instance axis (temporal module: 4
bands/core over seq T=64; band module: 8 frames/core over seq K=32); batch
B=4 stays together. X is exchanged with an AllToAll at the three internal
module boundaries; groupnorm partial stats ride along in the exchanged
buffer. The last band module's per-core output slices are assembled on the
host.

GroupNorm folding: gamma is folded into W_ih on the host (W' = W_ih
diag(gamma)), so the device only needs the per-batch scale rs_b (applied
once to the imported X) and a per-module additive gate correction
corr = W_ih@beta + b_ih + b_hh - m_b * W_ih@gamma, assembled on-device from
two uploaded static tensors (base, -W@gamma) and the runtime scalar
m_b = mu_b*rs_b. No normalized-X staging pass and no separate input
projection pass exist: each step's gate PSUM accumulates
[16 W'_ih@x-column matmuls] + [identity@corr deposit] + [16 W_hh@h matmuls]
directly (PE matmuls are cheap; this removes all DVE/Act staging work).

The skip path reads straight out of the A2A receive buffer via a strided
view (layer-0 temporal reads the raw uploaded x).

Step chain per direction: PSUM gates -> one sigmoid (g-gate rows doubled on
the host so tanh(g)=2*sigmoid(2g)-1 rides the same op) -> t2=f*c (gpsimd) |
t1=(A-0.5)*i (DVE) -> c=2*t1+t2 (DVE) -> tanh(c) (Act) -> h=o*tanh (gpsimd).
DVE program order is t1_d, c_d per direction so c never queues behind the
other direction's t1.

Precision: weights, h, corr, and the A2A payload in bf16; PSUM/c in fp32.
"""

import numpy as np

import concourse.bass as bass  # noqa: F401
import concourse.bacc as bacc
import concourse.mybir as mybir
from concourse import tile
from concourse.bass_utils import run_bass_kernel_spmd

R = 8            # cores
B = 4            # batch
K = 32           # bands (KMOD)
T = 64           # frames
N = 128          # features
G = 512          # 4N gates
L = 2            # layers
EPS = 1e-5
NT = K // R      # temporal instances per core
NB = T // R      # band instances per core
M_STAT = K * T * N
CW = 136         # per-destination chunk: 128 data cols + 8 stats cols

FP = mybir.dt.float32
WDT = mybir.dt.bfloat16
HDT = WDT

GATE_PERM = [0, 1, 3, 2]  # torch (i,f,g,o) -> slots (i,f,o,g)


# ---------------------------------------------------------------------------
# device graph
# ---------------------------------------------------------------------------

def build_nc():
    nc = bacc.Bacc(None, target_bir_lowering=False, debug=False, num_devices=R)

    xs = nc.dram_tensor("xs", [N, NT, T, B], FP, kind="ExternalInput")
    xsc0 = nc.dram_tensor("xsc0", [N, NT, T, B], WDT, kind="ExternalInput")
    corr0 = nc.dram_tensor("corr0", [N, 2, 4, NT, B], WDT,
                           kind="ExternalInput")
    twi = nc.dram_tensor("twi", [L, N, NT, 2, G], WDT, kind="ExternalInput")
    twh = nc.dram_tensor("twh", [L, N, NT, 2, G], HDT, kind="ExternalInput")
    tbase = nc.dram_tensor("tbase", [L, N, 2, 4, NT], FP,
                           kind="ExternalInput")
    twg = nc.dram_tensor("twg", [L, N, 2, 4, NT], FP, kind="ExternalInput")
    tfcw = nc.dram_tensor("tfcw", [L, N, NT, 2, N], WDT, kind="ExternalInput")
    tfcb = nc.dram_tensor("tfcb", [L, N, NT], FP, kind="ExternalInput")
    bwi = nc.dram_tensor("bwi", [L, N, NB, 2, G], WDT, kind="ExternalInput")
    bwh = nc.dram_tensor("bwh", [L, N, NB, 2, G], HDT, kind="ExternalInput")
    bbase = nc.dram_tensor("bbase", [L, N, 2, 4, NB], FP,
                           kind="ExternalInput")
    bwg = nc.dram_tensor("bwg", [L, N, 2, 4, NB], FP, kind="ExternalInput")
    bfcw = nc.dram_tensor("bfcw", [L, N, NB, 2, N], WDT, kind="ExternalInput")
    bfcb = nc.dram_tensor("bfcb", [L, N, NB], FP, kind="ExternalInput")
    out_ext = nc.dram_tensor("out", [N, NB, K, B], FP, kind="ExternalOutput")

    rg = [list(range(R))]

    with tile.TileContext(nc) as tc:
        with (
            tc.tile_pool(name="dram", bufs=1, space="DRAM") as dram,
            tc.tile_pool(name="const", bufs=1) as cpool,
            tc.tile_pool(name="wt", bufs=2) as wtp,
            tc.tile_pool(name="wb", bufs=2) as wbp,
            tc.tile_pool(name="xp", bufs=1) as xp,
            tc.tile_pool(name="hcp", bufs=2) as hcp,
            tc.tile_pool(name="gts", bufs=24) as gts,
            tc.tile_pool(name="smp", bufs=2) as smp,
            tc.tile_pool(name="cst", bufs=1) as cst,
            tc.tile_pool(name="ps_g", bufs=2, space="PSUM") as ps_g,
            tc.tile_pool(name="ps_f", bufs=3, space="PSUM") as ps_f,
            tc.tile_pool(name="ps_s", bufs=1, space="PSUM") as ps_s,
        ):
            # A2A bounce/output buffers for the 3 internal boundaries
            bnc = [dram.tile([R, N, CW], WDT, name=f"bnc{i}") for i in range(3)]
            gth = [dram.tile([R, N, CW], WDT, name=f"gth{i}") for i in range(3)]

            ones = cpool.tile([N, N], WDT)
            nc.vector.memset(ones[:], 1.0)
            from concourse.masks import make_identity
            ident = cpool.tile([N, N], WDT)
            make_identity(nc, ident[:])
            zeros = cpool.tile([N, B], HDT)
            nc.vector.memset(zeros[:], 0.0)
            eps_t = cpool.tile([N, 1], FP)
            nc.vector.memset(eps_t[:], float(EPS))
            dmy = cpool.tile([N, 1], FP)

            def act_preload():
                # tiny sigmoid forces the sigmoid/tanh act-func-set load NOW
                # (off the critical path) instead of right before the next
                # real sigmoid; the dbg DMA keeps it live through DCE
                nc.scalar.activation(dmy[:], eps_t[:],
                                     mybir.ActivationFunctionType.Sigmoid)

            act_preload()

            # ---------------- helpers ----------------

            def stats_prep(gst):
                """gst [N, R, 8] partial moments (already /M on the sender)
                -> (rs [N,B], m [N,B])."""
                ps = ps_s.tile([N, 8], FP, tag="ps_tot")
                for r in range(R):
                    nc.tensor.matmul(ps[:], ones[:], gst[:, r],
                                     start=(r == 0), stop=(r == R - 1))
                mom = smp.tile([N, 8], FP, tag="st_mom")
                nc.vector.tensor_copy(mom[:], ps[:])
                msq = smp.tile([N, 4], FP, tag="st_msq")
                nc.vector.tensor_tensor(msq[:], mom[:, 0:4], mom[:, 0:4],
                                        op=mybir.AluOpType.mult)
                var = smp.tile([N, 4], FP, tag="st_var")
                nc.vector.tensor_tensor(var[:], mom[:, 4:8], msq[:],
                                        op=mybir.AluOpType.subtract)
                sd = smp.tile([N, 4], FP, tag="st_sd")
                nc.scalar.activation(sd[:], var[:],
                                     mybir.ActivationFunctionType.Sqrt,
                                     bias=eps_t[:, 0:1])
                rs = smp.tile([N, 4], FP, tag="st_rs")
                nc.vector.reciprocal(rs[:], sd[:])
                m = smp.tile([N, 4], FP, tag="st_m")
                nc.vector.tensor_tensor(m[:], mom[:, 0:4], rs[:],
                                        op=mybir.AluOpType.mult)
                return rs, m

            def make_corr(base_t, wg_t, m, ni, tag):
                """corr[N, 2, 4, ni, B] (bf16) = base + m_b * (-W@gamma).
                base/wg uploaded as [N, 2, 4, ni] so (c j) merges to keep
                every operand <=3D for the verifier."""
                corr = smp.tile([N, 2, 4, ni, B], WDT, tag=tag)
                wgv = wg_t[:].rearrange("p d c j -> p d (c j)")
                bsv = base_t[:].rearrange("p d c j -> p d (c j)")
                for b in range(B):
                    nc.vector.scalar_tensor_tensor(
                        corr[:, :, :, :, b].rearrange("p d c j -> p d (c j)"),
                        wgv, m[:, b:b + 1], bsv,
                        op0=mybir.AluOpType.mult, op1=mybir.AluOpType.add)
                return corr

            def data_view(gsb, kind):
                """Received [N, R, CW] -> 5-dim X view (no copy).
                t: [p, kk(inst), q, tt, b] with seq t=(q,tt);
                b: [p, q, kk, tt(inst), b] with seq k=(q,kk)."""
                d = gsb[:, :, 0:128]
                if kind == "t":
                    return d.rearrange("p q (kk tt b) -> p kk q tt b",
                                       kk=NT, tt=NB)
                return d.rearrange("p q (kk tt b) -> p q kk tt b",
                                   kk=NT, tt=NB)

            def skip_slicer(xsk, kind):
                """skip_of(j) -> dense [N, S, B] instance view."""
                if kind == "t":
                    return lambda j: xsk[:, j]
                return lambda j: xsk[:, :, j]

            def import_scale(gsb, rs, kind, tagk, tagc):
                """Materialize recv data (dense bf16 skip tile) and the
                rs-scaled copy, per seq range with outer quarters first.
                Copies on DVE (middle on Act); scales alternate DVE/Act
                per batch so neither engine serializes the whole chain."""
                shape = [N, NT, T, B] if kind == "t" else [N, K, NB, B]
                w = NB if kind == "t" else NT
                xsk = xp.tile(shape, WDT, tag=tagk)
                xsc = xp.tile(shape, WDT, tag=tagc)
                d = gsb[:, :, 0:128]
                for lo, hi in ((0, 2), (6, 8), (2, 6)):
                    src = d[:, lo:hi].rearrange(
                        "p q (kk ttb) -> p kk q ttb" if kind == "t"
                        else "p q (kk ttb) -> p q kk ttb", kk=NT)
                    if kind == "t":
                        dst = xsk[:, :, lo * NB:hi * NB, :].rearrange(
                            "p kk (q tt) b -> p kk q (tt b)", tt=NB)
                    else:
                        dst = xsk[:, lo * NT:hi * NT, :, :].rearrange(
                            "p (q kk) tt b -> p q kk (tt b)", kk=NT)
                    if lo == 2:
                        nc.scalar.activation(
                            dst, src, mybir.ActivationFunctionType.Copy)
                    else:
                        nc.vector.tensor_copy(dst, src)
                    # one broadcast multiply per range (all batches): a
                    # single writer per region keeps each step matmul's
                    # wait list at one semaphore instead of four
                    if kind == "t":
                        dv = xsc[:, :, lo * w:hi * w, :]
                        sv = xsk[:, :, lo * w:hi * w, :]
                    else:
                        dv = xsc[:, lo * w:hi * w, :, :]
                        sv = xsk[:, lo * w:hi * w, :, :]
                    rsb = (rs[:].unsqueeze(1).unsqueeze(1)
                           .broadcast_to(list(dv.shape)))
                    nc.vector.tensor_tensor(dv, sv, rsb,
                                            op=mybir.AluOpType.mult)
                return xsk, xsc

            def module(kind, xsc, skip_of, corr, wi_t, wh_t, fw_t, fb_t,
                       contrib, out_dma=None, cmaj=False):
                """One module. contrib: [N, NT, T, B] (t), [N, K, NB, B]
                (band), or [N, NB, K, B] (band with cmaj=True).
                skip_of(j) -> dense [N, S, B] view."""
                NI = NT if kind == "t" else NB
                S = T if kind == "t" else K
                W = NI * B

                def inst(ap, j):  # [N, S, B] contrib view for instance j
                    return ap[:, j] if (kind == "t" or cmaj) else ap[:, :, j]

                def xcol(j, pos):  # [N, B] input column
                    return (xsc[:, j, pos] if kind == "t"
                            else xsc[:, pos, j])

                h_dir = [hcp.tile([N, S, NI, B], HDT, tag=f"h{d}",
                                  name=f"h{d}") for d in range(2)]
                c_st = [cst.tile([N, 2, W], FP, tag=f"cs{p}", name=f"cs{p}")
                        for p in range(2)]
                nc.vector.memset(c_st[1][:], 0.0)

                def fc_range(j, p0, p1):
                    pf = ps_f.tile([N, p1 - p0, B], FP, tag="pf", name="pf")
                    for d in range(2):
                        nc.tensor.matmul(pf[:], fw_t[:, j, d],
                                         h_dir[d][:, p0:p1, j, :],
                                         start=(d == 0), stop=(d == 1))
                    nc.vector.scalar_tensor_tensor(
                        inst(contrib, j)[:, p0:p1, :], pf[:],
                        fb_t[:, j:j + 1], skip_of(j)[:, p0:p1, :],
                        op0=mybir.AluOpType.add, op1=mybir.AluOpType.add)

                def quarters(ap):
                    """contiguous [p, S, B] -> [p, 2, S/4*B]: outer
                    quarters via a step-slice on the quarter axis."""
                    return (ap.rearrange("p (x tt) b -> p x tt b",
                                         tt=S // 4)
                            .rearrange("p x tt b -> p x (tt b)")[:, 0:4:3])

                def fc_outer(j):
                    """FC for both outer quarters off one PSUM group; one
                    merged stt for temporal (contiguous views), two for
                    band (strided instance views can't merge (tt b))."""
                    q = S // 4
                    pf = ps_f.tile([N, 2, q, B], FP, tag="pf", name="pf")
                    first = True
                    for d in range(2):
                        for hh, (r0, r1) in enumerate(((0, q), (S - q, S))):
                            nc.tensor.matmul(pf[:, hh], fw_t[:, j, d],
                                             h_dir[d][:, r0:r1, j, :],
                                             start=first,
                                             stop=(d == 1 and hh == 1))
                            first = False
                    if kind == "t":
                        nc.vector.scalar_tensor_tensor(
                            quarters(inst(contrib, j)),
                            pf[:].rearrange("p x tt b -> p x (tt b)"),
                            fb_t[:, j:j + 1], quarters(skip_of(j)),
                            op0=mybir.AluOpType.add,
                            op1=mybir.AluOpType.add)
                    else:
                        for hh, (r0, r1) in enumerate(((0, q), (S - q, S))):
                            nc.vector.scalar_tensor_tensor(
                                inst(contrib, j)[:, r0:r1, :], pf[:, hh],
                                fb_t[:, j:j + 1], skip_of(j)[:, r0:r1, :],
                                op0=mybir.AluOpType.add,
                                op1=mybir.AluOpType.add)

                for s in range(S):
                    acts = {}
                    for d in range(2):
                        pos = s if d == 0 else S - 1 - s
                        pg = ps_g.tile([N, 4, NI, B], FP, tag=f"pg{d}",
                                       name=f"pg{d}")
                        # start=True on the first matmul zeroes the whole
                        # 2KB PSUM zero-region (the bank), so later matmuls
                        # accumulate-from-zero without per-region starts
                        first = True
                        for j in range(NI):
                            xc = xcol(j, pos)
                            for c in range(4):
                                nc.tensor.matmul(
                                    pg[:, c, j],
                                    wi_t[:, j, d, c * N:(c + 1) * N],
                                    xc, start=first, stop=False,
                                    skip_group_check=True)
                                first = False
                        nc.tensor.matmul(pg[:], ident[:], corr[:, d],
                                         start=False, stop=(s == 0),
                                         skip_group_check=True)
                        if s > 0:
                            for j in range(NI):
                                prv = s - 1 if d == 0 else S - s
                                hprev = h_dir[d][:, prv, j, :]
                                for c in range(4):
                                    nc.tensor.matmul(
                                        pg[:, c, j],
                                        wh_t[:, j, d, c * N:(c + 1) * N],
                                        hprev, start=False,
                                        stop=(j == NI - 1 and c == 3),
                                        skip_group_check=True)
                        act = gts.tile([N, 4, W], FP, tag=f"act{d}",
                                       name=f"act{d}")
                        pgv = pg[:].rearrange("p c j b -> p c (j b)")
                        # g-chunk weights/corr are pre-doubled on the host, so
                        # one sigmoid covers all four chunks; tanh(g)=2*A-1
                        nc.scalar.activation(act[:], pgv[:],
                                             mybir.ActivationFunctionType.Sigmoid)
                        acts[d] = act
                    # c = 2*(A-0.5)*i + f*c_prev; t2/t1/c back-to-back on DVE
                    # (no cross-engine hop), grouped per direction so c_d
                    # never queues behind the other direction's ops
                    for d in range(2):
                        t2 = gts.tile([N, W], FP, tag=f"t2{d}", name=f"t2{d}")
                        nc.vector.tensor_tensor(t2[:], acts[d][:, 1],
                                                c_st[1 - s % 2][:, d],
                                                op=mybir.AluOpType.mult)
                        t1 = gts.tile([N, W], FP, tag=f"t1{d}", name=f"t1{d}")
                        nc.vector.scalar_tensor_tensor(
                            t1[:], acts[d][:, 3], 0.5, acts[d][:, 0],
                            op0=mybir.AluOpType.subtract,
                            op1=mybir.AluOpType.mult)
                        nc.vector.scalar_tensor_tensor(
                            c_st[s % 2][:, d], t1[:], 2.0, t2[:],
                            op0=mybir.AluOpType.mult, op1=mybir.AluOpType.add)
                    tch = gts.tile([N, 2, W], FP, tag="tc")
                    for d in range(2):
                        nc.scalar.activation(tch[:, d], c_st[s % 2][:, d],
                                             mybir.ActivationFunctionType.Tanh)
                    for d in range(2):
                        pos = s if d == 0 else S - 1 - s
                        nc.gpsimd.tensor_tensor(
                            h_dir[d][:, pos].rearrange("p j b -> p (j b)"),
                            acts[d][:, 2], tch[:, d], op=mybir.AluOpType.mult)
                    # FC middle half dripped behind the recurrence
                    fj = s - (3 * S // 4 - 1)
                    if 0 <= fj < NI:
                        fc_range(fj, S // 4, 3 * S // 4)

                # FC + bias + skip for the outer quarters
                for j in range(NI):
                    fc_outer(j)
                    if out_dma is not None:
                        out_dma(j)

            def partial_stats(contrib):
                """[N, 8] per-partition moments/M (b: 0..3 mean part,
                4..7 E[x^2] part): 1/M folded into the activation scale
                (1/sqrt(M) for the squares; M = 2^18 so both are exact)."""
                stp = smp.tile([N, 8], FP, tag="stp")
                # squares+accum on Act; plain sums on DVE tensor_reduce so
                # the two halves run on different engines concurrently.
                # Separate scratch slices per op (a shared scratch would
                # WAW-chain the activations at full effect latency).
                scr = smp.tile([N, 4, 256], WDT, tag="scr")
                raw = smp.tile([N, 4], FP, tag="st_raw")
                for b in range(B):
                    cv = contrib[:, :, :, b]
                    svb = scr[:, b].rearrange("p (a s) -> p a s",
                                              a=cv.shape[1])
                    nc.vector.tensor_reduce(raw[:, b:b + 1], cv,
                                            axis=mybir.AxisListType.XY,
                                            op=mybir.AluOpType.add)
                    nc.scalar.activation(svb, cv,
                                         mybir.ActivationFunctionType.Square,
                                         scale=1.0 / (M_STAT ** 0.5),
                                         accum_out=stp[:, 4 + b:5 + b])
                nc.vector.tensor_scalar(stp[:, 0:4], raw[:], 1.0 / M_STAT,
                                        None, op0=mybir.AluOpType.mult)
                return stp

            def export_data(contrib, bt, kind):
                """contrib data cols -> bounce rows; DMAs don't wait on
                stats (those ride a separate small DMA)."""
                ex = xp.tile([N, R, CW], WDT, tag="ex")
                if kind == "t":
                    src = contrib[:].rearrange(
                        "p kk (q tt) b -> p q kk (tt b)", q=R)
                else:
                    src = contrib[:].rearrange(
                        "p (q kk) tt b -> p q kk (tt b)", q=R)
                nc.vector.tensor_copy(
                    ex[:, :, 0:128].rearrange("p q (kk tb) -> p q kk tb",
                                              kk=NT), src)
                for q4 in range(0, R, 4):
                    nc.sync.dma_start(
                        out=bt[q4:q4 + 4, :, 0:128].rearrange(
                            "q p c -> p q c"),
                        in_=ex[:, q4:q4 + 4, 0:128])
                return ex

            def export_stats(ex, stp, bt):
                nc.vector.tensor_copy(
                    ex[:, :, 128:],
                    stp[:].unsqueeze(1).broadcast_to([N, R, 8]))
                nc.sync.dma_start(
                    out=bt[:, :, 128:].rearrange("q p c -> p q c"),
                    in_=ex[:, :, 128:])

            def a2a(i):
                nc.gpsimd.collective_compute(
                    "AllToAll", mybir.AluOpType.bypass, replica_groups=rg,
                    ins=[bnc[i].opt()], outs=[gth[i].opt()])

            def import_g(i):
                """gathered [R, N, CW] -> SBUF; stats cols first so the
                scale/corr chain overlaps the big data import."""
                g = gth[i]
                gst = xp.tile([N, R, 8], WDT, tag=f"gst{i}", name=f"gst{i}")
                nc.sync.dma_start(
                    out=gst[:],
                    in_=g[:].rearrange("q p c -> p q c")[:, :, 128:])
                gsb = xp.tile([N, R, CW], WDT, tag=f"gsb{i}", name=f"gsb{i}")
                for k, q4 in enumerate(range(0, R, 4)):
                    eng = nc.scalar if k == 0 else nc.sync
                    eng.dma_start(
                        out=gsb[:, q4:q4 + 4],
                        in_=g[q4:q4 + 4].rearrange("q p c -> p q c"))
                return gsb, gst

            def load_t(li):
                wi = wtp.tile([N, NT, 2, G], WDT, tag="twi")
                wh = wtp.tile([N, NT, 2, G], HDT, tag="twh")
                fw = wtp.tile([N, NT, 2, N], WDT, tag="tfw")
                bs = wtp.tile([N, 2, 4, NT], FP, tag="tbs")
                wg = wtp.tile([N, 2, 4, NT], FP, tag="twgt")
                fb = wtp.tile([N, NT], FP, tag="tfb")
                for dst, src in ((wi, twi), (wh, twh), (fw, tfcw),
                                 (bs, tbase), (wg, twg), (fb, tfcb)):
                    nc.sync.dma_start(out=dst[:], in_=src[li])
                return wi, wh, fw, fb, bs, wg

            def load_b(li):
                wi = wbp.tile([N, NB, 2, G], WDT, tag="bwi")
                wh = wbp.tile([N, NB, 2, G], HDT, tag="bwh")
                fw = wbp.tile([N, NB, 2, N], WDT, tag="bfw")
                bs = wbp.tile([N, 2, 4, NB], FP, tag="bbs")
                wg = wbp.tile([N, 2, 4, NB], FP, tag="bwgt")
                fb = wbp.tile([N, NB], FP, tag="bfb")
                for dst, src in ((wi, bwi), (wh, bwh), (fw, bfcw),
                                 (bs, bbase), (wg, bwg), (fb, bfcb)):
                    nc.sync.dma_start(out=dst[:], in_=src[li])
                return wi, wh, fw, fb, bs, wg

            # ---------------- the 4 modules ----------------

            # T0: projections from uploaded rs-scaled x; corr precomputed.
            # Critical-path loads (xsc0, corr0, wi halves) split across the
            # SP and Activation HWDGE queues; the rest follows.
            xsc0_t = xp.tile([N, NT, T, B], WDT, tag="xsc0")
            nc.sync.dma_start(out=xsc0_t[:], in_=xsc0[:])
            corr0_t = smp.tile([N, 2, 4, NT, B], WDT, tag="corr0")
            nc.scalar.dma_start(out=corr0_t[:], in_=corr0[:])
            wi = wtp.tile([N, NT, 2, G], WDT, tag="twi")
            nc.sync.dma_start(out=wi[:, :, 0], in_=twi[0, :, :, 0])
            nc.scalar.dma_start(out=wi[:, :, 1], in_=twi[0, :, :, 1])
            wh = wtp.tile([N, NT, 2, G], HDT, tag="twh")
            nc.sync.dma_start(out=wh[:, :, 0], in_=twh[0, :, :, 0])
            nc.scalar.dma_start(out=wh[:, :, 1], in_=twh[0, :, :, 1])
            xs_t = xp.tile([N, NT, T, B], FP, tag="xs")
            nc.sync.dma_start(out=xs_t[:], in_=xs[:])
            fw = wtp.tile([N, NT, 2, N], WDT, tag="tfw")
            nc.scalar.dma_start(out=fw[:], in_=tfcw[0])
            fb = wtp.tile([N, NT], FP, tag="tfb")
            nc.scalar.dma_start(out=fb[:], in_=tfcb[0])
            # prefetch every later module's weights now: emitted here,
            # their DMAs run during the loops instead of queueing behind
            # boundary export DMAs on the in-order SP queue
            wB0 = load_b(0)
            wT1 = load_t(1)
            wB1 = load_b(1)
            ct = cst.tile([N, NT, T, B], FP, tag="ct")
            module("t", xsc0_t, lambda j: xs_t[:, j], corr0_t,
                   wi, wh, fw, fb, ct)
            ex = export_data(ct, bnc[0], "t")
            export_stats(ex, partial_stats(ct), bnc[0])
            a2a(0)

            # B0
            gsb, gst = import_g(0)
            wi, wh, fw, fb, bs, wg = wB0
            rs, m = stats_prep(gst)
            corr = make_corr(bs, wg, m, NB, "corrb")
            xsk, xb = import_scale(gsb, rs, "b", "xk0", "xb0")
            cb = cst.tile([N, K, NB, B], FP, tag="cb")
            module("b", xb, skip_slicer(xsk, "b"), corr, wi, wh, fw, fb, cb)
            ex = export_data(cb, bnc[1], "b")
            export_stats(ex, partial_stats(cb), bnc[1])
            a2a(1)

            # T1
            gsb, gst = import_g(1)
            wi, wh, fw, fb, bs, wg = wT1
            rs, m = stats_prep(gst)
            corr = make_corr(bs, wg, m, NT, "corrt")
            xsk, xt1 = import_scale(gsb, rs, "t", "xk1", "xt1")
            ct1 = cst.tile([N, NT, T, B], FP, tag="ct")
            module("t", xt1, skip_slicer(xsk, "t"), corr, wi, wh, fw, fb,
                   ct1)
            ex = export_data(ct1, bnc[2], "t")
            export_stats(ex, partial_stats(ct1), bnc[2])
            a2a(2)

            # B1: stream the output DMA per finished instance
            gsb, gst = import_g(2)
            wi, wh, fw, fb, bs, wg = wB1
            rs, m = stats_prep(gst)
            corr = make_corr(bs, wg, m, NB, "corrb")
            xsk, xb1 = import_scale(gsb, rs, "b", "xk2", "xb1")
            # instance-major contrib so each instance's rows are contiguous
            # and its output DMA streams right after its last FC
            cb1 = cst.tile([N, NB, K, B], FP, tag="cb1")

            def out_dma(j):
                # two half-size DMAs instead of eight per-instance ones:
                # fewer ~630ns SEQ holds ahead of the final transfer
                if j == 3:
                    nc.sync.dma_start(out=out_ext[:, 0:4], in_=cb1[:, 0:4])
                elif j == 6:
                    nc.scalar.dma_start(out=out_ext[:, 4:7],
                                        in_=cb1[:, 4:7])
                elif j == 7:
                    nc.sync.dma_start(out=out_ext[:, 7:8], in_=cb1[:, 7:8])

            module("b", xb1, skip_slicer(xsk, "b"), corr, wi, wh, fw, fb,
                   cb1, out_dma=out_dma, cmaj=True)

    nc.finalize()
    return nc


# ---------------------------------------------------------------------------
# host side
# ---------------------------------------------------------------------------

_NC_CACHE = {}


def _wdt_np(a):
    import ml_dtypes
    return np.asarray(a, dtype=ml_dtypes.bfloat16)


def _perm_double(w4):
    """w4 [..., 4, X]: apply GATE_PERM on axis -2 and double the g slot."""
    w4 = np.asarray(w4[..., GATE_PERM, :]).copy()
    w4[..., 3, :] *= 2.0
    return w4


def _prep_lstm_w(w, sl, gamma=None):
    """w: [L, I, 2, G, N] full -> lhsT [L, N, ni, 2, G] for instances sl.
    gamma [L, N] is folded in along the contraction axis when given."""
    ws = np.asarray(w)[:, sl].astype(np.float64)  # [L, ni, 2, G, N]
    ni = ws.shape[1]
    if gamma is not None:
        ws = ws * np.asarray(gamma, np.float64)[:, None, None, None, :]
    ws = ws.reshape(L, ni, 2, 4, N * N)
    ws = _perm_double(ws).reshape(L, ni, 2, 4, N, N)
    # (l, j, d, c, g, n) -> (l, n, j, d, c, g)
    ws = ws.transpose(0, 5, 1, 2, 3, 4).reshape(L, N, ni, 2, G)
    return np.ascontiguousarray(ws.astype(np.float32))


def _prep_base_wg(w_ih, b_ih, b_hh, beta, gamma, sl):
    """base = perm/doubled (W@beta + b_ih + b_hh); wgneg = -perm/doubled
    W@gamma. Returns [L, N, ni, 2, 4] fp32 each."""
    w = np.asarray(w_ih)[:, sl].astype(np.float64)      # [L, ni, 2, G, N]
    ni = w.shape[1]
    wb = np.einsum("ljdgn,ln->ljdg", w, np.asarray(beta, np.float64))
    wg = np.einsum("ljdgn,ln->ljdg", w, np.asarray(gamma, np.float64))
    bsum = (np.asarray(b_ih)[:, sl] + np.asarray(b_hh)[:, sl]).astype(
        np.float64)                                     # [L, ni, 2, G]
    base = (wb + bsum).reshape(L, ni, 2, 4, N)
    wgn = (-wg).reshape(L, ni, 2, 4, N)
    base = _perm_double(base)
    wgn = _perm_double(wgn)
    # (l, j, d, c, g) -> (l, g, d, c, j)
    base = base.transpose(0, 4, 2, 3, 1)
    wgn = wgn.transpose(0, 4, 2, 3, 1)
    return (np.ascontiguousarray(base.astype(np.float32)),
            np.ascontiguousarray(wgn.astype(np.float32)))


def _prep_fcw(w, sl):
    ws = np.asarray(w)[:, sl]                     # [L, ni, N, 2N]
    ni = ws.shape[1]
    ws = ws.reshape(L, ni, N, 2, N)
    # (l, j, no, d, dk) -> (l, dk, j, d, no)
    return np.ascontiguousarray(ws.transpose(0, 4, 1, 3, 2))


def kernel(x, tw_ih, tw_hh, tb_ih, tb_hh, tfc_w, tfc_b, tgn_g, tgn_b,
           bw_ih, bw_hh, bb_ih, bb_hh, bfc_w, bfc_b, bgn_g, bgn_b):
    x = np.asarray(x, dtype=np.float32)

    if "nc" not in _NC_CACHE:
        _NC_CACHE["nc"] = build_nc()
    nc = _NC_CACHE["nc"]

    # layer-0 temporal groupnorm: exact host stats
    mu = x.mean(axis=(1, 2, 3))                    # [B]
    var = x.var(axis=(1, 2, 3))                    # [B]
    rs0 = 1.0 / np.sqrt(var + EPS)                 # [B]
    m0 = mu * rs0                                  # [B]
    xsc0_full = x * rs0[:, None, None, None]       # [B, K, T, N]

    tgn_g = np.asarray(tgn_g, np.float32)
    tgn_b = np.asarray(tgn_b, np.float32)
    bgn_g = np.asarray(bgn_g, np.float32)
    bgn_b = np.asarray(bgn_b, np.float32)

    in_maps = []
    for r in range(R):
        slt = slice(4 * r, 4 * r + 4)
        slb = slice(8 * r, 8 * r + 8)
        xs = np.ascontiguousarray(
            x[:, slt].transpose(3, 1, 2, 0)).astype(np.float32)  # [N,NT,T,B]
        xsc0 = np.ascontiguousarray(
            xsc0_full[:, slt].transpose(3, 1, 2, 0))             # [N,NT,T,B]
        tbase_r, twg_r = _prep_base_wg(tw_ih, tb_ih, tb_hh,
                                       tgn_b, tgn_g, slt)
        bbase_r, bwg_r = _prep_base_wg(bw_ih, bb_ih, bb_hh,
                                       bgn_b, bgn_g, slb)
        # corr0 = base + m_b * wgneg for layer-0 temporal: [N, 2, 4, NT, B]
        corr0_r = np.ascontiguousarray(
            tbase_r[0][..., None]
            + twg_r[0][..., None] * m0[None, None, None, :])
        tfcb_r = np.ascontiguousarray(
            np.asarray(tfc_b)[:, slt].transpose(0, 2, 1)).astype(np.float32)
        bfcb_r = np.ascontiguousarray(
            np.asarray(bfc_b)[:, slb].transpose(0, 2, 1)).astype(np.float32)
        in_maps.append({
            "xs": xs,
            "xsc0": _wdt_np(xsc0),
            "corr0": _wdt_np(corr0_r),
            "twi": _wdt_np(_prep_lstm_w(tw_ih, slt, gamma=tgn_g)),
            "twh": _wdt_np(_prep_lstm_w(tw_hh, slt)),
            "tbase": tbase_r,
            "twg": twg_r,
            "tfcw": _wdt_np(_prep_fcw(tfc_w, slt)),
            "tfcb": tfcb_r,
            "bwi": _wdt_np(_prep_lstm_w(bw_ih, slb, gamma=bgn_g)),
            "bwh": _wdt_np(_prep_lstm_w(bw_hh, slb)),
            "bbase": bbase_r,
            "bwg": bwg_r,
            "bfcw": _wdt_np(_prep_fcw(bfc_w, slb)),
            "bfcb": bfcb_r,
        })

    global _LAST_IN_MAPS
    _LAST_IN_MAPS = in_maps
    res = run_bass_kernel_spmd(nc, in_maps, core_ids=list(range(R)))
    outs = res.results

    full = np.zeros((B, K, T, N), dtype=np.float32)
    for r in range(R):
        o = np.asarray(outs[r]["out"]).reshape(N, NB, K, B)
        full[:, :, 8 * r:8 * r + 8, :] = o.transpose(3, 2, 1, 0)
    return full[:, :K - 2]
